# revision 1
# baseline (speedup 1.0000x reference)
"""Trainium2 Bass kernel for nn_MultiHeadAttention (B=4, S=2048, D=512, H=8, DH=64).

Sharding: 8 cores = 4 batches x 2 query-halves. Each core computes full
attention for all 8 heads over its 1024 query rows (K/V projections are
duplicated within a batch pair). The output is a pure concatenation.

Per-core pipeline (bf16 datapath, fp32 PSUM accumulation):
  1. Inputs/weights are pre-cast to bf16 on the host; X^T (feature-major)
     loads straight from HBM via DMA xbar transpose.
  2. Project: Q^T, K^T feature-major ([512, S]); V natural ([S, 512]) with an
     extra all-ones column appended per head (65-col layout).
  3. Attention per head, per 128-row k-block:
       S^T[k, q] = K^T_h(stationary) @ Q^T_h   (contraction = d_head 64)
       P^T = exp(S^T / 8)                       (ScalarE, PSUM -> SBUF bf16)
       z^T[65, q] += [V_h | 1](stationary) @ P^T  (row 64 = softmax denom)
  4. Normalize: reciprocal of row 64, broadcast, multiply -> Z^T.
  5. Output projection from Z^T + bias in fp32, DMA out.
"""

import os
import sys

import numpy as np

sys.path.insert(0, "/opt/trn_rl_repo")

import ml_dtypes
import concourse.bacc as bacc
import concourse.bass as bass
import concourse.mybir as mybir
import concourse.tile as tile
from concourse import bass_utils

F32 = mybir.dt.float32
BF16 = mybir.dt.bfloat16

B, S, D, H, DH = 4, 2048, 512, 8, 64
SQ = S // 2          # query rows per core
NKB = S // 128       # 16 k-blocks
NFT = D // 128       # 4 feature tiles
NQB = SQ // 128      # 8 query blocks
N_CORES = 8

Exp = mybir.ActivationFunctionType.Exp
Identity = mybir.ActivationFunctionType.Identity


def build_program(dbg=False):
    nc = bacc.Bacc("TRN2", target_bir_lowering=False, debug=False)
    dbg_out = {}
    if dbg:
        for nm, shp, dt in [("KT0", [128, S], BF16), ("QT0", [128, SQ], BF16),
                            ("VA0", [128, 520], BF16), ("PA00", [128, SQ], BF16),
                            ("ZA0", [65, SQ], F32), ("ZT0", [128, SQ], BF16),
                            ("XKT0", [128, S], BF16)]:
            dbg_out[nm] = nc.dram_tensor(nm, shp, dt, kind="ExternalOutput").ap()

    xq = nc.dram_tensor("XQ", [SQ, D], BF16, kind="ExternalInput").ap()
    xk = nc.dram_tensor("XK", [S, D], BF16, kind="ExternalInput").ap()
    xv = nc.dram_tensor("XV", [S, D], BF16, kind="ExternalInput").ap()
    wq = nc.dram_tensor("Wq", [D, D], BF16, kind="ExternalInput").ap()
    wk = nc.dram_tensor("Wk", [D, D], BF16, kind="ExternalInput").ap()
    wv = nc.dram_tensor("Wv", [D, D], BF16, kind="ExternalInput").ap()
    wo = nc.dram_tensor("Wo", [D, D], BF16, kind="ExternalInput").ap()
    bq = nc.dram_tensor("bq", [D, 1], F32, kind="ExternalInput").ap()
    bk = nc.dram_tensor("bk", [D, 1], F32, kind="ExternalInput").ap()
    bv = nc.dram_tensor("bv", [1, D], F32, kind="ExternalInput").ap()
    bo = nc.dram_tensor("bo", [1, D], F32, kind="ExternalInput").ap()
    out = nc.dram_tensor("OUT", [SQ, D], F32, kind="ExternalOutput").ap()

    from contextlib import ExitStack

    with tile.TileContext(nc) as tc, ExitStack() as ctx:
        const = ctx.enter_context(tc.tile_pool(name="const", bufs=1))
        xt_pool = ctx.enter_context(tc.tile_pool(name="xt", bufs=1))
        w_pool = ctx.enter_context(tc.tile_pool(name="w", bufs=1))
        kt_pool = ctx.enter_context(tc.tile_pool(name="kt", bufs=1))
        qt_pool = ctx.enter_context(tc.tile_pool(name="qt", bufs=1))
        v_pool = ctx.enter_context(tc.tile_pool(name="v", bufs=1))
        p_pool = ctx.enter_context(tc.tile_pool(name="p", bufs=23))
        zt_pool = ctx.enter_context(tc.tile_pool(name="zt", bufs=1))
        nrm_pool = ctx.enter_context(tc.tile_pool(name="nrm", bufs=3))
        out_pool = ctx.enter_context(tc.tile_pool(name="outp", bufs=2))

        # One PSUM pool, 4 tags x 2 banks = all 8 banks.  Prefix (projection)
        # and epilogue tiles rotate through the same tags that attention uses
        # for sA/sB/zA/zB.
        ps = ctx.enter_context(tc.tile_pool(name="ps", bufs=1, space="PSUM"))
        ps_ctr = [0, 0]

        def ps_tile(shape, tags, name):
            i = 0 if tags == "ab" else 1
            tag = ("a", "b", "c", "d")[2 * i + ps_ctr[i] % 2]
            ps_ctr[i] += 1
            return ps.tile(shape, F32, tag=tag, name=name, padded_shape=[128, SQ])

        # ---- weights: one DMA per tensor, sliced into 4 contraction chunks ----
        def load_w(wdram, name, eng):
            big = w_pool.tile([128, NFT * D], BF16, tag=f"w{name}", name=f"w{name}")
            eng.dma_start(
                big[:].rearrange("p (g c) -> p g c", g=NFT),
                wdram.rearrange("(g p) c -> p g c", p=128),
            )
            return [big[:, D * mc:D * (mc + 1)] for mc in range(NFT)]

        # ---- X^T via DMA xbar transpose ----
        def load_xt(xdram, nrows, name, engs):
            tiles = []
            for ft in range(NFT):
                t = xt_pool.tile([128, nrows], BF16, tag=f"xt{name}{ft}",
                                 name=f"xt{name}{ft}", padded_shape=[128, S])
                engs[ft % len(engs)].dma_start(
                    t[:], xdram[:, 128 * ft:128 * (ft + 1)], transpose=True
                )
                tiles.append(t)
            return tiles

        # ---- DMA loads, ordered by first use ----
        wk_t = load_w(wk, "k", nc.sync)
        xkt = load_xt(xk, S, "k", [nc.sync])
        bq_all = const.tile([128, NFT], F32, tag="bqa")
        nc.sync.dma_start(
            bq_all[:].rearrange("p (g o) -> p g o", g=NFT),
            bq.rearrange("(g p) o -> p g o", p=128),
        )
        bk_all = const.tile([128, NFT], F32, tag="bka")
        nc.sync.dma_start(
            bk_all[:].rearrange("p (g o) -> p g o", g=NFT),
            bk.rearrange("(g p) o -> p g o", p=128),
        )
        bq_t = [bq_all[:, ft:ft + 1] for ft in range(NFT)]
        bk_t = [bk_all[:, ft:ft + 1] for ft in range(NFT)]
        bv_row = const.tile([1, D], F32, tag="bvr")
        nc.sync.dma_start(bv_row[:], bv[:])
        bv_bc = const.tile([128, D], F32, tag="bvb")
        nc.gpsimd.partition_broadcast(bv_bc[:], bv_row[:], channels=128)
        bo_row = const.tile([1, D], F32, tag="bor")
        nc.sync.dma_start(bo_row[:], bo[:])
        bo_bc = const.tile([128, D], F32, tag="bob")
        nc.gpsimd.partition_broadcast(bo_bc[:], bo_row[:], channels=128)

        wq_t = load_w(wq, "q", nc.sync)
        xqt = load_xt(xq, SQ, "q", [nc.sync])
        wv_t = load_w(wv, "v", nc.sync)
        xvt = load_xt(xv, S, "v", [nc.sync])
        wo_t = load_w(wo, "o", nc.sync)

        k_t = [kt_pool.tile([128, S], BF16, tag=f"kt{ft}", name=f"kt{ft}")
               for ft in range(NFT)]
        q_t = [qt_pool.tile([128, SQ], BF16, tag=f"qt{ft}", name=f"qt{ft}")
               for ft in range(NFT)]

        def proj_k_chunk(ft, sc):
            pj = ps_tile([128, 1024], "cd", f"pjk{ft}{sc}")
            for h2 in range(2):
                for mc in range(NFT):
                    nc.tensor.matmul(
                        pj[:, 512 * h2:512 * (h2 + 1)],
                        wk_t[mc][:, 128 * ft:128 * (ft + 1)],
                        xkt[mc][:, 1024 * sc + 512 * h2:1024 * sc + 512 * (h2 + 1)],
                        start=(mc == 0),
                        stop=(mc == NFT - 1),
                    )
            nc.any.tensor_scalar_add(
                k_t[ft][:, 1024 * sc:1024 * (sc + 1)], pj[:], bk_t[ft][:],
            )

        def proj_q_chunk(ft):
            pj = ps_tile([128, 1024], "cd", f"pjq{ft}")
            for h2 in range(2):
                for mc in range(NFT):
                    nc.tensor.matmul(
                        pj[:, 512 * h2:512 * (h2 + 1)],
                        wq_t[mc][:, 128 * ft:128 * (ft + 1)],
                        xqt[mc][:, 512 * h2:512 * (h2 + 1)],
                        start=(mc == 0),
                        stop=(mc == NFT - 1),
                    )
            nc.any.tensor_scalar_add(q_t[ft][:], pj[:], bq_t[ft][:])

        def proj_kq(ft):
            proj_k_chunk(ft, 0)
            proj_q_chunk(ft)
            proj_k_chunk(ft, 1)

        # warm the ScalarE Exp table before the critical path (the first
        # ACTIVATE otherwise pays the ~2.7us ACT_TABLE_LOAD inline)
        warm = nrm_pool.tile([1, 8], F32, tag="warm")
        nc.gpsimd.memset(warm[:], 0.0)
        warm2 = nrm_pool.tile([1, 8], F32, tag="warm2")
        nc.scalar.activation(warm2[:], warm[:], Exp, scale=0.125)

        # ---- slot-scheduled emission ----------------------------------
        # PE is the binding engine; emit its work as one interleaved stream:
        #  - S + exp for (pair, kb) runs in slot (pair, kb)
        #  - V projections ride in pair-0 slots (PSUM c/d tags)
        #  - each pair's z-accumulation is deferred while c/d is busy, then
        #    drains two-groups-per-slot once its zA/zB tiles pin c/d
        #  - K/Q projections for pair p+1 slot into the c/d window between
        #    norm(p-1) and z(p) pinning
        proj_k_chunk(0, 0)
        proj_q_chunk(0)

        VW = H * (DH + 1)  # 520: per head 64 value cols + 1 ones col
        v_aug = [v_pool.tile([128, VW], BF16, tag=f"v{kb}", name=f"v{kb}")
                 for kb in range(NKB)]

        def v_group(kb):
            nc.gpsimd.memset(
                v_aug[kb][:].rearrange("p (h c) -> p h c", h=H)[:, :, DH:DH + 1],
                1.0,
            )
            pj = ps_tile([128, 512], "cd", f"pjv{kb}")
            for mc in range(NFT):
                nc.tensor.matmul(
                    pj[:],
                    xvt[mc][:, 128 * kb:128 * (kb + 1)],
                    wv_t[mc][:],
                    start=(mc == 0),
                    stop=(mc == NFT - 1),
                )
            nc.any.tensor_add(
                v_aug[kb][:].rearrange("p (h c) -> p h c", h=H)[:, :, 0:DH],
                pj[:].rearrange("p (h c) -> p h c", h=H),
                bv_bc[:].rearrange("p (h c) -> p h c", h=H),
            )

        z_t = [zt_pool.tile([128, SQ], BF16, tag=f"zt{p}", name=f"zt{p}")
               for p in range(NFT)]
        p_slabs = {}
        z_tiles = {}

        def s_exp(pair, kb):
            sA = ps.tile([128, SQ], F32, tag="a", name=f"sA{pair}_{kb}")
            sB = ps.tile([128, SQ], F32, tag="b", name=f"sB{pair}_{kb}")
            for qc in range(SQ // 512):
                qs = slice(512 * qc, 512 * (qc + 1))
                nc.tensor.matmul(
                    sA[:, qs],
                    k_t[pair][0:DH, 128 * kb:128 * (kb + 1)],
                    q_t[pair][0:DH, qs],
                    start=True, stop=True,
                    tile_position=(0, 0),
                )
                nc.tensor.matmul(
                    sB[:, qs],
                    k_t[pair][DH:128, 128 * kb:128 * (kb + 1)],
                    q_t[pair][DH:128, qs],
                    start=True, stop=True,
                    tile_position=(64, 0),
                )
            pA = p_pool.tile([128, SQ], BF16, tag="p", name=f"pA{pair}_{kb}")
            nc.scalar.activation(pA[:], sA[:], Exp, scale=0.125)
            pB = p_pool.tile([128, SQ], BF16, tag="p", name=f"pB{pair}_{kb}")
            nc.scalar.activation(pB[:], sB[:], Exp, scale=0.125)
            if dbg and pair == 0 and kb == 0:
                nc.sync.dma_start(dbg_out["PA00"][:], pA[:])
            p_slabs[(pair, kb)] = (pA, pB)

        def z_alloc(pair):
            zA = ps.tile([DH + 1, SQ], F32, tag="c", name=f"zA{pair}",
                         padded_shape=[128, SQ])
            zB = ps.tile([DH + 1, SQ], F32, tag="d", name=f"zB{pair}",
                         padded_shape=[128, SQ])
            z_tiles[pair] = (zA, zB)

        def z_group(pair, kb):
            zA, zB = z_tiles[pair]
            pA, pB = p_slabs.pop((pair, kb))
            hA, hB = 2 * pair, 2 * pair + 1
            for qc in range(SQ // 512):
                qs = slice(512 * qc, 512 * (qc + 1))
                nc.tensor.matmul(
                    zA[:, qs],
                    v_aug[kb][:, 65 * hA:65 * hA + 65],
                    pA[:, qs],
                    start=(kb == 0), stop=(kb == NKB - 1),
                    skip_group_check=True,
                )
                nc.tensor.matmul(
                    zB[:, qs],
                    v_aug[kb][:, 65 * hB:65 * hB + 65],
                    pB[:, qs],
                    start=(kb == 0), stop=(kb == NKB - 1),
                    skip_group_check=True,
                )

        def norm(pair):
            zA, zB = z_tiles.pop(pair)
            if dbg and pair == 0:
                zdump = out_pool.tile([65, SQ], F32, tag="zdump")
                nc.vector.tensor_copy(zdump[:], zA[:])
                nc.sync.dma_start(dbg_out["ZA0"][:], zdump[:])
            for z_ps, half in ((zA, 0), (zB, 1)):
                # custom-DVE recip mis-reads PSUM: stage the row via ScalarE
                rowc = nrm_pool.tile([1, SQ], F32, tag="rowc")
                nc.scalar.activation(rowc[:], z_ps[DH:DH + 1, :],
                                     mybir.ActivationFunctionType.Copy)
                recip = nrm_pool.tile([1, SQ], F32, tag="recip")
                nc.vector.reciprocal_approx_fast(recip[:], rowc[:])
                rbc = nrm_pool.tile([DH, SQ], F32, tag="rbc")
                nc.gpsimd.partition_broadcast(rbc[:], recip[:], channels=DH)
                nc.vector.tensor_mul(
                    z_t[pair][64 * half:64 * half + 64, :], z_ps[0:DH, :], rbc[:]
                )

        # slot schedule: slot (p, kb) -> extra emissions after S+exp
        feeder = [lambda: proj_k_chunk(0, 1),
                  lambda: proj_k_chunk(1, 0),
                  lambda: proj_q_chunk(1),
                  lambda: proj_k_chunk(1, 1)]
        feeder += [(lambda k: (lambda: v_group(k)))(kb) for kb in range(NKB)]
        fi = [0]

        def feed(n):
            for _ in range(n):
                if fi[0] < len(feeder):
                    feeder[fi[0]]()
                    fi[0] += 1

        for pair in range(NFT):
            for kb in range(NKB):
                s_exp(pair, kb)
                if pair == 0:
                    feed(2 if kb < 4 else 1)
                    if kb == NKB - 1:
                        feed(len(feeder))
                elif pair == 1:
                    if kb == 0:
                        z_alloc(0)
                    if kb < 8:
                        z_group(0, 2 * kb)
                        z_group(0, 2 * kb + 1)
                    elif kb == 8:
                        norm(0)
                    elif kb == 9:
                        proj_kq(2)
                    elif kb == 12:
                        z_alloc(1)
                    if kb >= 12:
                        z_group(1, 2 * (kb - 12))
                        z_group(1, 2 * (kb - 12) + 1)
                elif pair == 2:
                    if kb < 4:
                        z_group(1, 8 + 2 * kb)
                        z_group(1, 8 + 2 * kb + 1)
                    elif kb == 4:
                        norm(1)
                    elif kb == 5:
                        proj_kq(3)
                    elif kb == 8:
                        z_alloc(2)
                    if kb >= 8:
                        z_group(2, 2 * (kb - 8))
                        z_group(2, 2 * (kb - 8) + 1)
                else:
                    if kb == 0:
                        norm(2)
                    elif kb == 1:
                        z_alloc(3)
                    if kb >= 1:
                        z_group(3, kb - 1)
            if pair == NFT - 1:
                z_group(3, 15)
                norm(3)

        if dbg:
            nc.sync.dma_start(dbg_out["KT0"][:], k_t[0][:])
            nc.sync.dma_start(dbg_out["QT0"][:], q_t[0][:])
            nc.sync.dma_start(dbg_out["VA0"][:], v_aug[0][:])
            nc.sync.dma_start(dbg_out["XKT0"][:], xkt[0][:])
            nc.sync.dma_start(dbg_out["ZT0"][:], z_t[0][:])

        # ---- output projection ----
        for qb in range(NQB):
            po = ps_tile([128, D], "ab", f"po{qb}")
            for p4 in range(NFT):
                nc.tensor.matmul(
                    po[:],
                    z_t[p4][:, 128 * qb:128 * (qb + 1)],
                    wo_t[p4][:],
                    start=(p4 == 0),
                    stop=(p4 == NFT - 1),
                )
            ot = out_pool.tile([128, D], F32, tag="ot")
            nc.any.tensor_add(ot[:], po[:], bo_bc[:])
            nc.sync.dma_start(out[128 * qb:128 * (qb + 1), :], ot[:])

    nc.compile()
    return nc


_NC = None
LAST_RESULTS = None


def _get_nc():
    global _NC
    if _NC is None:
        _NC = build_program(dbg=bool(int(os.environ.get("KERNEL_DEBUG", "0"))))
    return _NC


def _bf(x):
    return np.ascontiguousarray(np.asarray(x).astype(ml_dtypes.bfloat16))


def kernel(Q, K, V, Wq, bq, Wk, bk, Wv, bv, Wo, bo):
    global LAST_RESULTS
    nc = _get_nc()
    Qb, Kb, Vb = _bf(Q), _bf(K), _bf(V)
    shared = {
        "Wq": _bf(Wq),
        "Wk": _bf(Wk),
        "Wv": _bf(Wv),
        "Wo": _bf(Wo),
        "bq": np.ascontiguousarray(np.asarray(bq, np.float32).reshape(D, 1)),
        "bk": np.ascontiguousarray(np.asarray(bk, np.float32).reshape(D, 1)),
        "bv": np.ascontiguousarray(np.asarray(bv, np.float32).reshape(1, D)),
        "bo": np.ascontiguousarray(np.asarray(bo, np.float32).reshape(1, D)),
    }
    in_maps = []
    for c in range(N_CORES):
        b, qh = c // 2, c % 2
        in_maps.append({
            "XQ": np.ascontiguousarray(Qb[b, SQ * qh:SQ * (qh + 1)]),
            "XK": Kb[b],
            "XV": Vb[b],
            **shared,
        })
    trace = bool(int(os.environ.get("KERNEL_TRACE", "0")))
    res = bass_utils.run_bass_kernel_spmd(
        nc, in_maps, core_ids=list(range(N_CORES)), trace=trace,
    )
    LAST_RESULTS = res
    out = np.empty((B, S, D), dtype=np.float32)
    for c in range(N_CORES):
        b, qh = c // 2, c % 2
        out[b, SQ * qh:SQ * (qh + 1)] = res.results[c]["OUT"]
    return out



# revision 8
# speedup vs baseline: 1.1433x; 1.1433x over previous
"""Trainium2 Bass kernel for nn_MultiHeadAttention (B=4, S=2048, D=512, H=8, DH=64).

Sharding: 8 cores = 4 batches x 2 query-halves. Each core computes full
attention for all 8 heads over its 1024 query rows (K/V projections are
duplicated within a batch pair). The output is a pure concatenation.

v2 datapath (natural-layout z + ScalarE-exclusive exp, all bf16):
  1. Host pre-transposes X to feature-major bf16 [D, S]; all loads are
     plain strided DMAs - no on-chip DMA transposes in the prologue.
  2. QKV projections in bf16 (optionally Q/K via fp8 DoubleRow with
     KERNEL_FP8QK=1; V path must stay bf16 for accuracy).
  3. Scores: bf16 dual-64 tile_position packing, S^T[k, q] per head pair.
  4. exp runs on ScalarE only (the binding engine, ~1.07us/[128,1024]
     tile); probs stay bf16 [k, q] per (pair-branch, kb).
  5. z accumulation in NATURAL layout: z[q, 130] += P_kb^T @ [VA|1|VB|1]
     per q-block: stationary = p-slice [128,128] (FWL fast weight load),
     moving = v_aug[kb] slice, N=65 per head -> ~2x fewer PE cycles than
     the old 65-partition z^T form. Denominators land at cols 64/129.
  6. Normalize = per-partition reciprocal columns + tensor_scalar muls
     (DVE); z then DMA-transposed (idle engines) to feature-major for
     the output projection.
  7. Output projection emitted transposed (out^T [D, SQ]) so the bias is
     per-partition; host un-transposes when assembling the result.

PSUM budget (8 banks): tags a, b = score tiles (2 banks each); tags
c0..c3 = one bank each, time-shared between projection pj chunks and the
per-pair zn accumulator (2 q-blocks per bank, chunk stride 256 floats).
"""

import os
import sys

import numpy as np

sys.path.insert(0, "/opt/trn_rl_repo")

import ml_dtypes
import concourse.bacc as bacc
import concourse.bass as bass
import concourse.mybir as mybir
import concourse.tile as tile
from concourse import bass_utils

F32 = mybir.dt.float32
BF16 = mybir.dt.bfloat16
FP8 = mybir.dt.float8e4
DR = mybir.MatmulPerfMode.DoubleRow

B, S, D, H, DH = 4, 2048, 512, 8, 64
SQ = S // 2          # query rows per core
NKB = S // 128       # 16 k-blocks
NFT = D // 128       # 4 feature tiles
NQB = SQ // 128      # 8 query blocks
N_CORES = 8

VW = 2 * DH + 2      # 130: [V_A | 1 | V_B | 1]
ZCH = 256            # zn chunk stride in floats (1KB); 2 chunks per bank

Exp = mybir.ActivationFunctionType.Exp

FP8QK = bool(int(os.environ.get("KERNEL_FP8QK", "0")))
WSCALE = 32.0 if FP8QK else 1.0
EXP_SCALE = 0.125 / (WSCALE * WSCALE)


def build_program(dbg=False):
    nc = bacc.Bacc("TRN2", target_bir_lowering=False, debug=False)
    dbg_out = {}
    if dbg:
        for nm, shp, dt in [("KT0", [128, S], BF16), ("QT0", [128, SQ], BF16),
                            ("VA0", [128, NFT * VW], BF16),
                            ("PA00", [128, SQ], BF16),
                            ("ZN0", [128, 2 * ZCH], F32),
                            ("ZT0", [128, SQ], BF16)]:
            dbg_out[nm] = nc.dram_tensor(nm, shp, dt, kind="ExternalOutput").ap()

    xdt = FP8 if FP8QK else BF16
    xqt = nc.dram_tensor("XQT", [D, SQ], xdt, kind="ExternalInput").ap()
    xkt = nc.dram_tensor("XKT", [D, S], xdt, kind="ExternalInput").ap()
    xvt = nc.dram_tensor("XVT", [D, S], BF16, kind="ExternalInput").ap()
    wq = nc.dram_tensor("WQP", [D, D], xdt, kind="ExternalInput").ap()
    wk = nc.dram_tensor("WKP", [D, D], xdt, kind="ExternalInput").ap()
    wv = nc.dram_tensor("WVP", [D, D], BF16, kind="ExternalInput").ap()
    wo = nc.dram_tensor("WOP", [D, D], BF16, kind="ExternalInput").ap()
    bq = nc.dram_tensor("bq", [D, 1], F32, kind="ExternalInput").ap()
    bk = nc.dram_tensor("bk", [D, 1], F32, kind="ExternalInput").ap()
    bv = nc.dram_tensor("bv", [1, D], F32, kind="ExternalInput").ap()
    bo = nc.dram_tensor("bo", [D, 1], F32, kind="ExternalInput").ap()
    out = nc.dram_tensor("OUTT", [D, SQ], F32, kind="ExternalOutput").ap()

    from contextlib import ExitStack

    with tile.TileContext(nc) as tc, ExitStack() as ctx:
        const = ctx.enter_context(tc.tile_pool(name="const", bufs=1))
        xt_pool = ctx.enter_context(tc.tile_pool(name="xt", bufs=1))
        w_pool = ctx.enter_context(tc.tile_pool(name="w", bufs=1))
        kt_pool = ctx.enter_context(tc.tile_pool(name="kt", bufs=1))
        qt_pool = ctx.enter_context(tc.tile_pool(name="qt", bufs=1))
        v_pool = ctx.enter_context(tc.tile_pool(name="v", bufs=1))
        p_pool = ctx.enter_context(tc.tile_pool(name="p", bufs=23))
        zt_pool = ctx.enter_context(tc.tile_pool(name="zt", bufs=1))
        nrm_pool = ctx.enter_context(tc.tile_pool(name="nrm", bufs=2))
        out_pool = ctx.enter_context(tc.tile_pool(name="outp", bufs=2))

        ps = ctx.enter_context(tc.tile_pool(name="ps", bufs=1, space="PSUM"))
        pj_ctr = [0]

        # pj chunks: [128, 512] f32 (1 bank), rotating tags c0..c3.
        def pj_tile(name):
            tag = f"c{pj_ctr[0] % 4}"
            pj_ctr[0] += 1
            return ps.tile([128, 512], F32, tag=tag, name=name,
                           padded_shape=[128, 512])

        # ---- SBUF allocations ------------------------------------------
        xq_s = xt_pool.tile([128, NFT * SQ], xdt, tag="xq", name="xq")
        xk_s = xt_pool.tile([128, NFT * S], xdt, tag="xk", name="xk")
        xv_s = xt_pool.tile([128, NFT * S], BF16, tag="xv", name="xv")
        wq_s = w_pool.tile([128, NFT * D], xdt, tag="wq", name="wq")
        wk_s = w_pool.tile([128, NFT * D], xdt, tag="wk", name="wk")
        wv_s = w_pool.tile([128, NFT * D], BF16, tag="wv", name="wv")
        wo_s = w_pool.tile([128, NFT * D], BF16, tag="wo", name="wo")

        def dma_chunked(sbuf_tile, dram):
            nc.sync.dma_start(
                sbuf_tile[:].rearrange("p (g c) -> p g c", g=NFT),
                dram.rearrange("(g p) c -> p g c", p=128),
            )

        def x3(t):
            return t[:].rearrange("p (g c) -> p g c", g=NFT)

        # ---- DMA loads, ordered by first use ----
        dma_chunked(wk_s, wk)
        dma_chunked(xk_s, xkt)
        bq_all = const.tile([128, NFT], F32, tag="bqa")
        nc.sync.dma_start(
            bq_all[:].rearrange("p (g o) -> p g o", g=NFT),
            bq.rearrange("(g p) o -> p g o", p=128),
        )
        bk_all = const.tile([128, NFT], F32, tag="bka")
        nc.sync.dma_start(
            bk_all[:].rearrange("p (g o) -> p g o", g=NFT),
            bk.rearrange("(g p) o -> p g o", p=128),
        )
        bo_all = const.tile([128, NFT], F32, tag="boa")
        nc.sync.dma_start(
            bo_all[:].rearrange("p (g o) -> p g o", g=NFT),
            bo.rearrange("(g p) o -> p g o", p=128),
        )
        bv_row = const.tile([1, D], F32, tag="bvr")
        nc.sync.dma_start(bv_row[:], bv[:])
        bv_bc = const.tile([128, D], F32, tag="bvb")
        nc.gpsimd.partition_broadcast(bv_bc[:], bv_row[:], channels=128)

        dma_chunked(wq_s, wq)
        dma_chunked(xq_s, xqt)
        dma_chunked(wv_s, wv)
        dma_chunked(xv_s, xvt)
        dma_chunked(wo_s, wo)

        k_t = [kt_pool.tile([128, S], BF16, tag=f"kt{ft}", name=f"kt{ft}")
               for ft in range(NFT)]
        q_t = [qt_pool.tile([128, SQ], BF16, tag=f"qt{ft}", name=f"qt{ft}")
               for ft in range(NFT)]
        bq_t = [bq_all[:, ft:ft + 1] for ft in range(NFT)]
        bk_t = [bk_all[:, ft:ft + 1] for ft in range(NFT)]

        # Q/K projection: one pj chunk [128, 512] per N-half.
        def proj_chunk(w_s, x_s, dst, bias, ft, sc):
            for h2 in range(2):
                lo = 1024 * sc + 512 * h2
                pj = pj_tile(f"pj{ft}{sc}{h2}")
                if FP8QK:
                    for kp in range(2):
                        nc.tensor.matmul(
                            pj[:],
                            x3(w_s)[:, 2 * kp:2 * kp + 2,
                                    128 * ft:128 * (ft + 1)],
                            x3(x_s)[:, 2 * kp:2 * kp + 2, lo:lo + 512],
                            start=(kp == 0), stop=(kp == 1),
                            perf_mode=DR,
                        )
                else:
                    for mc in range(NFT):
                        nc.tensor.matmul(
                            pj[:],
                            x3(w_s)[:, mc, 128 * ft:128 * (ft + 1)],
                            x3(x_s)[:, mc, lo:lo + 512],
                            start=(mc == 0), stop=(mc == NFT - 1),
                        )
                nc.vector.tensor_scalar_add(
                    dst[:, lo:lo + 512], pj[:], bias)

        def proj_k_chunk(ft, sc):
            proj_chunk(wk_s, xk_s, k_t[ft], bk_t[ft], ft, sc)

        def proj_q_chunk(ft):
            proj_chunk(wq_s, xq_s, q_t[ft], bq_t[ft], ft, 0)

        def proj_kq(ft):
            proj_k_chunk(ft, 0)
            proj_q_chunk(ft)
            proj_k_chunk(ft, 1)

        # warm the ScalarE Exp table before the critical path
        warm = nrm_pool.tile([1, 8], F32, tag="warm")
        nc.gpsimd.memset(warm[:], 0.0)
        warm2 = nrm_pool.tile([1, 8], F32, tag="warm2")
        nc.scalar.activation(warm2[:], warm[:], Exp, scale=EXP_SCALE)

        # ---- slot-scheduled emission ----------------------------------
        proj_k_chunk(0, 0)
        proj_q_chunk(0)

        # v_aug[kb]: all 4 head pairs side by side, each [V_A |1| V_B |1].
        v_aug = [v_pool.tile([128, NFT * VW], BF16, tag=f"v{kb}",
                             name=f"v{kb}")
                 for kb in range(NKB)]

        def v_group(kb):
            va4 = (v_aug[kb][:]
                   .rearrange("p (pr c) -> p pr c", c=VW)
                   .rearrange("p pr (h c) -> p pr h c", c=DH + 1))
            nc.gpsimd.memset(va4[:, :, :, DH:DH + 1], 1.0)
            pj = pj_tile(f"pjv{kb}")
            for mc in range(NFT):
                nc.tensor.matmul(
                    pj[:],
                    x3(xv_s)[:, mc, 128 * kb:128 * (kb + 1)],
                    x3(wv_s)[:, mc, :],
                    start=(mc == 0), stop=(mc == NFT - 1),
                )
            nc.vector.tensor_add(
                va4[:, :, :, 0:DH],
                pj[:].rearrange("p (pr h c) -> p pr h c", pr=NFT, h=2),
                bv_bc[:].rearrange("p (pr h c) -> p pr h c", pr=NFT, h=2),
            )

        zt_nat = [zt_pool.tile([128, SQ], BF16, tag=f"zn{p}", name=f"ztn{p}")
                  for p in range(NFT)]
        z_tt = [zt_pool.tile([128, SQ], BF16, tag=f"zt{p}", name=f"ztt{p}")
                for p in range(NFT)]
        p_slabs = {}
        z_acc = {}

        def s_exp(pair, kb):
            pA = p_pool.tile([128, SQ], BF16, tag="p", name=f"pA{pair}_{kb}")
            pB = p_pool.tile([128, SQ], BF16, tag="p", name=f"pB{pair}_{kb}")
            sA = ps.tile([128, SQ], F32, tag="a", name=f"sA{pair}_{kb}",
                         padded_shape=[128, SQ])
            sB = ps.tile([128, SQ], F32, tag="b", name=f"sB{pair}_{kb}",
                         padded_shape=[128, SQ])
            for qc in range(SQ // 512):
                qs = slice(512 * qc, 512 * (qc + 1))
                nc.tensor.matmul(
                    sA[:, qs],
                    k_t[pair][0:DH, 128 * kb:128 * (kb + 1)],
                    q_t[pair][0:DH, qs],
                    start=True, stop=True,
                    tile_position=(0, 0),
                )
            nc.scalar.activation(pA[:], sA[:], Exp, scale=EXP_SCALE)
            for qc in range(SQ // 512):
                qs = slice(512 * qc, 512 * (qc + 1))
                nc.tensor.matmul(
                    sB[:, qs],
                    k_t[pair][DH:128, 128 * kb:128 * (kb + 1)],
                    q_t[pair][DH:128, qs],
                    start=True, stop=True,
                    tile_position=(64, 0),
                )
            nc.scalar.activation(pB[:], sB[:], Exp, scale=EXP_SCALE)
            if dbg and pair == 0 and kb == 0:
                nc.sync.dma_start(dbg_out["PA00"][:], pA[:])
            p_slabs[(pair, kb)] = (pA, pB)

        def z_alloc(pair):
            z_acc[pair] = [
                ps.tile([128, 2 * ZCH], F32, tag=f"c{t}", name=f"zn{pair}_{t}",
                        padded_shape=[128, 2 * ZCH])
                for t in range(4)
            ]
            # full-bank memset: transfers byte ownership from the pj tiles
            # that previously lived in these banks (clean WAR edges)
            for t in range(4):
                nc.vector.memset(z_acc[pair][t][:], 0.0)

        # z natural: per q-block, zn[q, base:base+130] += p-slice^T @ v_aug.
        # start=True clears has_written for the WHOLE bank, so only the
        # first chunk written to each bank may use it; the bank-mates at
        # kb==0 rely on cleared bits -> overwrite semantics.
        def z_group(pair, kb):
            zn = z_acc[pair]
            pA, pB = p_slabs.pop((pair, kb))
            vA = v_aug[kb][:, VW * pair:VW * pair + VW]
            for qb in range(NQB):
                t, base = qb // 2, ZCH * (qb % 2)
                nc.tensor.matmul(
                    zn[t][:, base:base + DH + 1],
                    pA[:, 128 * qb:128 * (qb + 1)],
                    vA[:, 0:DH + 1],
                    start=(kb == 0 and qb % 2 == 0),
                    stop=(kb == NKB - 1),
                    skip_group_check=True,
                )
                nc.tensor.matmul(
                    zn[t][:, base + DH + 1:base + VW],
                    pB[:, 128 * qb:128 * (qb + 1)],
                    vA[:, DH + 1:VW],
                    start=False, stop=(kb == NKB - 1),
                    skip_group_check=True,
                )

        # normalize: denominators are cols 64 / 129 of each 130-col chunk
        def norm(pair):
            zn = z_acc.pop(pair)
            if dbg and pair == 0:
                zdump = out_pool.tile([128, 2 * ZCH], F32, tag="zdump")
                nc.vector.tensor_copy(zdump[:], zn[0][:])
                nc.sync.dma_start(dbg_out["ZN0"][:], zdump[:])
            den = nrm_pool.tile([128, 16], F32, tag=f"den{pair % 2}")
            for t in range(4):
                nc.vector.tensor_copy(
                    den[:, 4 * t:4 * t + 4].rearrange(
                        "p (j h) -> p j h", h=2),
                    zn[t][:].rearrange("p (j c) -> p j c", c=ZCH)
                    [:, :, DH:2 * DH + 2:DH + 1],
                )
            rec = nrm_pool.tile([128, 16], F32, tag=f"rec{pair % 2}")
            nc.vector.reciprocal_approx_fast(rec[:], den[:])
            for qb in range(NQB):
                t, base = qb // 2, ZCH * (qb % 2)
                for h in range(2):
                    nc.vector.tensor_scalar_mul(
                        zt_nat[pair][:, 128 * qb + 64 * h:
                                     128 * qb + 64 * (h + 1)],
                        zn[t][:, base + (DH + 1) * h:
                              base + (DH + 1) * h + DH],
                        rec[:, 2 * qb + h:2 * qb + h + 1],
                    )
            # transpose normalized z to feature-major via DMA (idle engines)
            for qb in range(NQB):
                nc.sync.dma_start(
                    z_tt[pair][:, 128 * qb:128 * (qb + 1)],
                    zt_nat[pair][:, 128 * qb:128 * (qb + 1)],
                    transpose=True,
                )

        # slot schedule: feeders ride the PE slack of pair-0 slots
        feeder = [lambda: proj_k_chunk(0, 1),
                  lambda: proj_k_chunk(1, 0),
                  lambda: proj_q_chunk(1),
                  lambda: proj_k_chunk(1, 1)]
        feeder += [(lambda k: (lambda: v_group(k)))(kb) for kb in range(NKB)]
        fi = [0]

        def feed(n):
            for _ in range(n):
                if fi[0] < len(feeder):
                    feeder[fi[0]]()
                    fi[0] += 1

        for pair in range(NFT):
            for kb in range(NKB):
                s_exp(pair, kb)
                if pair == 0:
                    feed(2 if kb < 4 else 1)
                    if kb == NKB - 1:
                        feed(len(feeder))
                elif pair == 1:
                    if kb == 0:
                        z_alloc(0)
                    if kb < 8:
                        z_group(0, 2 * kb)
                        z_group(0, 2 * kb + 1)
                    elif kb == 8:
                        norm(0)
                    elif kb == 9:
                        proj_kq(2)
                    elif kb == 12:
                        z_alloc(1)
                    if kb >= 12:
                        z_group(1, 2 * (kb - 12))
                        z_group(1, 2 * (kb - 12) + 1)
                elif pair == 2:
                    if kb < 4:
                        z_group(1, 8 + 2 * kb)
                        z_group(1, 8 + 2 * kb + 1)
                    elif kb == 4:
                        norm(1)
                    elif kb == 5:
                        proj_kq(3)
                    elif kb == 8:
                        z_alloc(2)
                    if kb >= 8:
                        z_group(2, 2 * (kb - 8))
                        z_group(2, 2 * (kb - 8) + 1)
                else:
                    if kb == 0:
                        norm(2)
                    elif kb == 1:
                        z_alloc(3)
                    if kb >= 1:
                        z_group(3, kb - 1)
            if pair == NFT - 1:
                z_group(3, 15)
                norm(3)

        if dbg:
            nc.sync.dma_start(dbg_out["KT0"][:], k_t[0][:])
            nc.sync.dma_start(dbg_out["QT0"][:], q_t[0][:])
            nc.sync.dma_start(dbg_out["VA0"][:], v_aug[0][:])
            nc.sync.dma_start(dbg_out["ZT0"][:], z_tt[0][:])

        # ---- output projection (transposed: out^T[fo, q]) ----
        for fo in range(NFT):
            po = ps.tile([128, SQ], F32, tag="a" if fo % 2 == 0 else "b",
                         name=f"po{fo}", padded_shape=[128, SQ])
            for qc in range(SQ // 512):
                qs = slice(512 * qc, 512 * (qc + 1))
                for p4 in range(NFT):
                    nc.tensor.matmul(
                        po[:, qs],
                        x3(wo_s)[:, p4, 128 * fo:128 * (fo + 1)],
                        z_tt[p4][:, qs],
                        start=(p4 == 0),
                        stop=(p4 == NFT - 1),
                    )
            ot = out_pool.tile([128, SQ], F32, tag="ot")
            nc.vector.tensor_scalar_add(ot[:], po[:], bo_all[:, fo:fo + 1])
            nc.sync.dma_start(out[128 * fo:128 * (fo + 1), :], ot[:])

    nc.compile()
    return nc


_NC = None
LAST_RESULTS = None


def _get_nc():
    global _NC
    if _NC is None:
        _NC = build_program(dbg=bool(int(os.environ.get("KERNEL_DEBUG", "0"))))
    return _NC


def _bf(x):
    return np.ascontiguousarray(np.asarray(x, np.float32).astype(
        ml_dtypes.bfloat16))


def _fp8(x):
    return np.ascontiguousarray(
        np.clip(np.asarray(x, np.float32), -240.0, 240.0).astype(
            ml_dtypes.float8_e4m3))


def make_in_maps(Q, K, V, Wq, bq, Wk, bk, Wv, bv, Wo, bo):
    Qf = np.asarray(Q, np.float32)
    Kf = np.asarray(K, np.float32)
    Vf = np.asarray(V, np.float32)
    cast_qk = _fp8 if FP8QK else _bf
    shared = {
        "WQP": cast_qk(np.asarray(Wq, np.float32) * WSCALE),
        "WKP": cast_qk(np.asarray(Wk, np.float32) * WSCALE),
        "WVP": _bf(Wv),
        "WOP": _bf(Wo),
        "bq": np.ascontiguousarray(
            np.asarray(bq, np.float32).reshape(D, 1) * WSCALE),
        "bk": np.ascontiguousarray(
            np.asarray(bk, np.float32).reshape(D, 1) * WSCALE),
        "bv": np.ascontiguousarray(np.asarray(bv, np.float32).reshape(1, D)),
        "bo": np.ascontiguousarray(np.asarray(bo, np.float32).reshape(D, 1)),
    }
    in_maps = []
    for c in range(N_CORES):
        b, qh = c // 2, c % 2
        in_maps.append({
            "XQT": cast_qk(Qf[b, SQ * qh:SQ * (qh + 1)].T),
            "XKT": cast_qk(Kf[b].T),
            "XVT": _bf(Vf[b].T),
            **shared,
        })
    return in_maps


def kernel(Q, K, V, Wq, bq, Wk, bk, Wv, bv, Wo, bo):
    global LAST_RESULTS
    nc = _get_nc()
    in_maps = make_in_maps(Q, K, V, Wq, bq, Wk, bk, Wv, bv, Wo, bo)
    trace = bool(int(os.environ.get("KERNEL_TRACE", "0")))
    res = bass_utils.run_bass_kernel_spmd(
        nc, in_maps, core_ids=list(range(N_CORES)), trace=trace,
    )
    LAST_RESULTS = res
    out = np.empty((B, S, D), dtype=np.float32)
    for c in range(N_CORES):
        b, qh = c // 2, c % 2
        out[b, SQ * qh:SQ * (qh + 1)] = res.results[c]["OUTT"].T
    return out


# revision 14
# speedup vs baseline: 1.1658x; 1.0197x over previous
"""Trainium2 Bass kernel for nn_MultiHeadAttention (B=4, S=2048, D=512, H=8, DH=64).

Sharding: 8 cores = 4 batches x 2 query-halves. Each core computes full
attention for all 8 heads over its 1024 query rows (K/V projections are
duplicated within a batch pair). The output is a pure concatenation.

v2 datapath (natural-layout z + ScalarE-exclusive exp, all bf16):
  1. Host pre-transposes X to feature-major bf16 [D, S]; all loads are
     plain strided DMAs - no on-chip DMA transposes in the prologue.
  2. QKV projections in bf16 (optionally Q/K via fp8 DoubleRow with
     KERNEL_FP8QK=1; V path must stay bf16 for accuracy).
  3. Scores: bf16 dual-64 tile_position packing, S^T[k, q] per head pair.
  4. exp runs on ScalarE only (the binding engine, ~1.07us/[128,1024]
     tile); probs stay bf16 [k, q] per (pair-branch, kb).
  5. z accumulation in NATURAL layout: z[q, 130] += P_kb^T @ [VA|1|VB|1]
     per q-block: stationary = p-slice [128,128] (FWL fast weight load),
     moving = v_aug[kb] slice, N=65 per head -> ~2x fewer PE cycles than
     the old 65-partition z^T form. Denominators land at cols 64/129.
  6. Normalize = per-partition reciprocal columns + tensor_scalar muls
     (DVE); z then DMA-transposed (idle engines) to feature-major for
     the output projection.
  7. Output projection emitted transposed (out^T [D, SQ]) so the bias is
     per-partition; host un-transposes when assembling the result.

PSUM budget (8 banks): tags a, b = score tiles (2 banks each); tags
c0..c3 = one bank each, time-shared between projection pj chunks and the
per-pair zn accumulator (2 q-blocks per bank, chunk stride 256 floats).
"""

import os
import sys

import numpy as np

sys.path.insert(0, "/opt/trn_rl_repo")

import ml_dtypes
import concourse.bacc as bacc
import concourse.bass as bass
import concourse.mybir as mybir
import concourse.tile as tile
from concourse import bass_utils

F32 = mybir.dt.float32
BF16 = mybir.dt.bfloat16
FP8 = mybir.dt.float8e4
DR = mybir.MatmulPerfMode.DoubleRow

B, S, D, H, DH = 4, 2048, 512, 8, 64
SQ = S // 2          # query rows per core
NKB = S // 128       # 16 k-blocks
NFT = D // 128       # 4 feature tiles
NQB = SQ // 128      # 8 query blocks
N_CORES = 8

VW = 2 * DH + 2      # 130: [V_A | 1 | V_B | 1]
ZCH = 256            # zn chunk stride in floats (1KB); 2 chunks per bank

Exp = mybir.ActivationFunctionType.Exp

FP8QK = bool(int(os.environ.get("KERNEL_FP8QK", "0")))
WSCALE = 32.0 if FP8QK else 1.0
EXP_SCALE = 0.125 / (WSCALE * WSCALE)


def build_program(dbg=False):
    nc = bacc.Bacc("TRN2", target_bir_lowering=False, debug=False)
    dbg_out = {}
    if dbg:
        for nm, shp, dt in [("KT0", [128, S], BF16), ("QT0", [128, SQ], BF16),
                            ("VA0", [128, NFT * VW], BF16),
                            ("PA00", [128, SQ], BF16),
                            ("ZN0", [128, 2 * ZCH], F32),
                            ("ZT0", [128, SQ], BF16)]:
            dbg_out[nm] = nc.dram_tensor(nm, shp, dt, kind="ExternalOutput").ap()

    xdt = FP8 if FP8QK else BF16
    xqt = nc.dram_tensor("XQT", [D, SQ], xdt, kind="ExternalInput").ap()
    xkt = nc.dram_tensor("XKT", [D, S], xdt, kind="ExternalInput").ap()
    xvt = nc.dram_tensor("XVT", [D, S], BF16, kind="ExternalInput").ap()
    wq = nc.dram_tensor("WQP", [D, D], xdt, kind="ExternalInput").ap()
    wk = nc.dram_tensor("WKP", [D, D], xdt, kind="ExternalInput").ap()
    wv = nc.dram_tensor("WVP", [D, D], BF16, kind="ExternalInput").ap()
    wo = nc.dram_tensor("WOP", [D, D], BF16, kind="ExternalInput").ap()
    bq = nc.dram_tensor("bq", [D, 1], F32, kind="ExternalInput").ap()
    bk = nc.dram_tensor("bk", [D, 1], F32, kind="ExternalInput").ap()
    bv = nc.dram_tensor("bv", [1, D], F32, kind="ExternalInput").ap()
    bo = nc.dram_tensor("bo", [D, 1], F32, kind="ExternalInput").ap()
    out = nc.dram_tensor("OUTT", [D, SQ], F32, kind="ExternalOutput").ap()

    from contextlib import ExitStack

    with tile.TileContext(nc) as tc, ExitStack() as ctx:
        const = ctx.enter_context(tc.tile_pool(name="const", bufs=1))
        xt_pool = ctx.enter_context(tc.tile_pool(name="xt", bufs=1))
        w_pool = ctx.enter_context(tc.tile_pool(name="w", bufs=1))
        kt_pool = ctx.enter_context(tc.tile_pool(name="kt", bufs=1))
        qt_pool = ctx.enter_context(tc.tile_pool(name="qt", bufs=1))
        v_pool = ctx.enter_context(tc.tile_pool(name="v", bufs=1))
        p_pool = ctx.enter_context(tc.tile_pool(name="p", bufs=26))
        zt_pool = ctx.enter_context(tc.tile_pool(name="zt", bufs=1))
        nrm_pool = ctx.enter_context(tc.tile_pool(name="nrm", bufs=2))
        out_pool = ctx.enter_context(tc.tile_pool(name="outp", bufs=2))

        ps = ctx.enter_context(tc.tile_pool(name="ps", bufs=1, space="PSUM"))
        pj_ctr = [0]

        # pj chunks: [128, 512] f32 (1 bank), rotating tags c0..c3.
        def pj_tile(name):
            tag = f"c{pj_ctr[0] % 4}"
            pj_ctr[0] += 1
            return ps.tile([128, 512], F32, tag=tag, name=name,
                           padded_shape=[128, 512])

        # ---- SBUF allocations ------------------------------------------
        xq_s = xt_pool.tile([128, NFT * SQ], xdt, tag="xq", name="xq")
        xk_s = xt_pool.tile([128, NFT * S], xdt, tag="xk", name="xk")
        xv_s = xt_pool.tile([128, NFT * S], BF16, tag="xv", name="xv")
        wq_s = w_pool.tile([128, NFT * D], xdt, tag="wq", name="wq")
        wk_s = w_pool.tile([128, NFT * D], xdt, tag="wk", name="wk")
        wv_s = w_pool.tile([128, NFT * D], BF16, tag="wv", name="wv")
        wo_s = w_pool.tile([128, NFT * D], BF16, tag="wo", name="wo")

        # split chunk loads across the two HWDGE queues so transfers
        # parallelize over DMA engines
        dma_rr = [0]

        def dma_chunked(sbuf_tile, dram):
            ncols = dram.shape[1]
            for g in range(NFT):
                eng = (nc.sync, nc.scalar)[dma_rr[0] % 2]
                dma_rr[0] += 1
                eng.dma_start(
                    sbuf_tile[:, g * ncols:(g + 1) * ncols],
                    dram[128 * g:128 * (g + 1), :],
                )

        def x3(t):
            return t[:].rearrange("p (g c) -> p g c", g=NFT)

        # ---- DMA loads, ordered by first use ----
        dma_chunked(wk_s, wk)
        dma_chunked(xk_s, xkt)
        bq_all = const.tile([128, NFT], F32, tag="bqa")
        nc.sync.dma_start(
            bq_all[:].rearrange("p (g o) -> p g o", g=NFT),
            bq.rearrange("(g p) o -> p g o", p=128),
        )
        bk_all = const.tile([128, NFT], F32, tag="bka")
        nc.sync.dma_start(
            bk_all[:].rearrange("p (g o) -> p g o", g=NFT),
            bk.rearrange("(g p) o -> p g o", p=128),
        )
        bo_all = const.tile([128, NFT], F32, tag="boa")
        nc.sync.dma_start(
            bo_all[:].rearrange("p (g o) -> p g o", g=NFT),
            bo.rearrange("(g p) o -> p g o", p=128),
        )
        bv_row = const.tile([1, D], F32, tag="bvr")
        nc.sync.dma_start(bv_row[:], bv[:])
        bv_bc = const.tile([128, D], F32, tag="bvb")
        nc.gpsimd.partition_broadcast(bv_bc[:], bv_row[:], channels=128)

        dma_chunked(wq_s, wq)
        dma_chunked(xq_s, xqt)
        dma_chunked(wv_s, wv)
        dma_chunked(xv_s, xvt)
        dma_chunked(wo_s, wo)

        k_t = [kt_pool.tile([128, S], BF16, tag=f"kt{ft}", name=f"kt{ft}")
               for ft in range(NFT)]
        q_t = [qt_pool.tile([128, SQ], BF16, tag=f"qt{ft}", name=f"qt{ft}")
               for ft in range(NFT)]
        bq_t = [bq_all[:, ft:ft + 1] for ft in range(NFT)]
        bk_t = [bk_all[:, ft:ft + 1] for ft in range(NFT)]

        # Q/K projection: one pj chunk [128, 512] per N-half.
        def proj_chunk(w_s, x_s, dst, bias, ft, sc):
            for h2 in range(2):
                lo = 1024 * sc + 512 * h2
                pj = pj_tile(f"pj{ft}{sc}{h2}")
                if FP8QK:
                    for kp in range(2):
                        nc.tensor.matmul(
                            pj[:],
                            x3(w_s)[:, 2 * kp:2 * kp + 2,
                                    128 * ft:128 * (ft + 1)],
                            x3(x_s)[:, 2 * kp:2 * kp + 2, lo:lo + 512],
                            start=(kp == 0), stop=(kp == 1),
                            perf_mode=DR,
                        )
                else:
                    for mc in range(NFT):
                        nc.tensor.matmul(
                            pj[:],
                            x3(w_s)[:, mc, 128 * ft:128 * (ft + 1)],
                            x3(x_s)[:, mc, lo:lo + 512],
                            start=(mc == 0), stop=(mc == NFT - 1),
                        )
                nc.vector.tensor_scalar_add(
                    dst[:, lo:lo + 512], pj[:], bias)

        def proj_k_chunk(ft, sc):
            proj_chunk(wk_s, xk_s, k_t[ft], bk_t[ft], ft, sc)

        def proj_q_chunk(ft):
            proj_chunk(wq_s, xq_s, q_t[ft], bq_t[ft], ft, 0)

        def proj_kq(ft):
            proj_k_chunk(ft, 0)
            proj_q_chunk(ft)
            proj_k_chunk(ft, 1)

        # warm the ScalarE Exp table before the critical path
        warm = nrm_pool.tile([1, 8], F32, tag="warm")
        nc.gpsimd.memset(warm[:], 0.0)
        warm2 = nrm_pool.tile([1, 8], F32, tag="warm2")
        nc.scalar.activation(warm2[:], warm[:], Exp, scale=EXP_SCALE)

        # ---- slot-scheduled emission ----------------------------------
        proj_k_chunk(0, 0)
        proj_q_chunk(0)

        # v_aug[kb]: all 4 head pairs side by side, each [V_A |1| V_B |1].
        v_aug = [v_pool.tile([128, NFT * VW], BF16, tag=f"v{kb}",
                             name=f"v{kb}")
                 for kb in range(NKB)]

        def v_group(kb):
            va4 = (v_aug[kb][:]
                   .rearrange("p (pr c) -> p pr c", c=VW)
                   .rearrange("p pr (h c) -> p pr h c", c=DH + 1))
            nc.gpsimd.memset(va4[:, :, :, DH:DH + 1], 1.0)
            pj = pj_tile(f"pjv{kb}")
            for mc in range(NFT):
                nc.tensor.matmul(
                    pj[:],
                    x3(xv_s)[:, mc, 128 * kb:128 * (kb + 1)],
                    x3(wv_s)[:, mc, :],
                    start=(mc == 0), stop=(mc == NFT - 1),
                )
            nc.vector.tensor_add(
                va4[:, :, :, 0:DH],
                pj[:].rearrange("p (pr h c) -> p pr h c", pr=NFT, h=2),
                bv_bc[:].rearrange("p (pr h c) -> p pr h c", pr=NFT, h=2),
            )

        zt_nat = [zt_pool.tile([128, SQ], BF16, tag=f"zn{p}", name=f"ztn{p}")
                  for p in range(NFT)]
        z_tt = [zt_pool.tile([128, SQ], BF16, tag=f"zt{p}", name=f"ztt{p}")
                for p in range(NFT)]
        p_slabs = {}
        z_acc = {}

        def s_exp(pair, kb):
            pA = p_pool.tile([128, SQ], BF16, tag="p", name=f"pA{pair}_{kb}")
            pB = p_pool.tile([128, SQ], BF16, tag="p", name=f"pB{pair}_{kb}")
            sA = ps.tile([128, SQ], F32, tag="a", name=f"sA{pair}_{kb}",
                         padded_shape=[128, SQ])
            sB = ps.tile([128, SQ], F32, tag="b", name=f"sB{pair}_{kb}",
                         padded_shape=[128, SQ])
            for qc in range(SQ // 512):
                qs = slice(512 * qc, 512 * (qc + 1))
                nc.tensor.matmul(
                    sA[:, qs],
                    k_t[pair][0:DH, 128 * kb:128 * (kb + 1)],
                    q_t[pair][0:DH, qs],
                    start=True, stop=True,
                    tile_position=(0, 0),
                )
            nc.scalar.activation(pA[:], sA[:], Exp, scale=EXP_SCALE)
            for qc in range(SQ // 512):
                qs = slice(512 * qc, 512 * (qc + 1))
                nc.tensor.matmul(
                    sB[:, qs],
                    k_t[pair][DH:128, 128 * kb:128 * (kb + 1)],
                    q_t[pair][DH:128, qs],
                    start=True, stop=True,
                    tile_position=(64, 0),
                )
            nc.scalar.activation(pB[:], sB[:], Exp, scale=EXP_SCALE)
            if dbg and pair == 0 and kb == 0:
                nc.sync.dma_start(dbg_out["PA00"][:], pA[:])
            p_slabs[(pair, kb)] = (pA, pB)

        def z_alloc(pair):
            z_acc[pair] = [
                ps.tile([128, 2 * ZCH], F32, tag=f"c{t}", name=f"zn{pair}_{t}",
                        padded_shape=[128, 2 * ZCH])
                for t in range(4)
            ]
            # full-bank memset: transfers byte ownership from the pj tiles
            # that previously lived in these banks (clean WAR edges)
            for t in range(4):
                nc.vector.memset(z_acc[pair][t][:], 0.0)

        # z natural: per q-block, zn[q, base:base+130] += p-slice^T @ v_aug.
        # start=True clears has_written for the WHOLE bank, so only the
        # first chunk written to each bank may use it; the bank-mates at
        # kb==0 rely on cleared bits -> overwrite semantics.
        def z_group(pair, kb):
            zn = z_acc[pair]
            pA, pB = p_slabs.pop((pair, kb))
            vA = v_aug[kb][:, VW * pair:VW * pair + VW]
            for qb in range(NQB):
                t, base = qb // 2, ZCH * (qb % 2)
                nc.tensor.matmul(
                    zn[t][:, base:base + DH + 1],
                    pA[:, 128 * qb:128 * (qb + 1)],
                    vA[:, 0:DH + 1],
                    start=(kb == 0 and qb % 2 == 0),
                    stop=(kb == NKB - 1),
                    skip_group_check=True,
                )
                nc.tensor.matmul(
                    zn[t][:, base + DH + 1:base + VW],
                    pB[:, 128 * qb:128 * (qb + 1)],
                    vA[:, DH + 1:VW],
                    start=False, stop=(kb == NKB - 1),
                    skip_group_check=True,
                )

        # normalize: denominators are cols 64 / 129 of each 130-col chunk
        def norm(pair):
            zn = z_acc.pop(pair)
            if dbg and pair == 0:
                zdump = out_pool.tile([128, 2 * ZCH], F32, tag="zdump")
                nc.vector.tensor_copy(zdump[:], zn[0][:])
                nc.sync.dma_start(dbg_out["ZN0"][:], zdump[:])
            den = nrm_pool.tile([128, 16], F32, tag=f"den{pair % 2}")
            for t in range(4):
                nc.vector.tensor_copy(
                    den[:, 4 * t:4 * t + 4].rearrange(
                        "p (j h) -> p j h", h=2),
                    zn[t][:].rearrange("p (j c) -> p j c", c=ZCH)
                    [:, :, DH:2 * DH + 2:DH + 1],
                )
            rec = nrm_pool.tile([128, 16], F32, tag=f"rec{pair % 2}")
            nc.vector.reciprocal_approx_fast(rec[:], den[:])
            for qb in range(NQB):
                t, base = qb // 2, ZCH * (qb % 2)
                for h in range(2):
                    nc.vector.tensor_scalar_mul(
                        zt_nat[pair][:, 128 * qb + 64 * h:
                                     128 * qb + 64 * (h + 1)],
                        zn[t][:, base + (DH + 1) * h:
                              base + (DH + 1) * h + DH],
                        rec[:, 2 * qb + h:2 * qb + h + 1],
                    )
            # transpose normalized z to feature-major via DMA. Dispatch is
            # ~1.2us each on the issuing queue: sync alone for pairs 0-2
            # (hidden under attention), sync+scalar for the last pair
            # (ScalarE is done with exps by then).
            for qb in range(NQB):
                eng = nc.scalar if (pair == NFT - 1 and qb % 2) else nc.sync
                eng.dma_start(
                    z_tt[pair][:, 128 * qb:128 * (qb + 1)],
                    zt_nat[pair][:, 128 * qb:128 * (qb + 1)],
                    transpose=True,
                )

        # slot schedule: feeders ride the PE slack of pair-0 slots
        feeder = [lambda: proj_k_chunk(0, 1),
                  lambda: proj_k_chunk(1, 0),
                  lambda: proj_q_chunk(1),
                  lambda: proj_k_chunk(1, 1)]
        feeder += [(lambda k: (lambda: v_group(k)))(kb) for kb in range(NKB)]
        fi = [0]

        def feed(n):
            for _ in range(n):
                if fi[0] < len(feeder):
                    feeder[fi[0]]()
                    fi[0] += 1

        # Per slot: z-groups and feeders FIRST (they have no dependency on
        # the in-flight exps), then the score matmuls (which stall on the
        # previous slot's exp draining the PSUM score tiles).
        for pair in range(NFT):
            for kb in range(NKB):
                if pair == 0:
                    feed(2 if kb < 4 else 1)
                    if kb == NKB - 1:
                        feed(len(feeder))
                elif pair == 1:
                    if kb == 0:
                        z_alloc(0)
                    if kb < 8:
                        z_group(0, 2 * kb)
                        z_group(0, 2 * kb + 1)
                    elif kb == 8:
                        norm(0)
                    elif kb == 9:
                        proj_kq(2)
                    elif kb == 12:
                        z_alloc(1)
                    if kb >= 12:
                        z_group(1, 2 * (kb - 12))
                        z_group(1, 2 * (kb - 12) + 1)
                elif pair == 2:
                    if kb < 4:
                        z_group(1, 8 + 2 * kb)
                        z_group(1, 8 + 2 * kb + 1)
                    elif kb == 4:
                        norm(1)
                    elif kb == 5:
                        proj_kq(3)
                    elif kb == 9:
                        z_alloc(2)
                    if kb >= 9:
                        z_group(2, 2 * (kb - 9))
                        z_group(2, 2 * (kb - 9) + 1)
                else:
                    if kb == 0:
                        z_group(2, 14)
                        z_group(2, 15)
                    elif kb == 1:
                        norm(2)
                    elif kb == 2:
                        z_alloc(3)
                    if kb >= 2:
                        z_group(3, kb - 2)
                s_exp(pair, kb)
            if pair == NFT - 1:
                z_group(3, 14)
                z_group(3, 15)
                norm(3)

        if dbg:
            nc.sync.dma_start(dbg_out["KT0"][:], k_t[0][:])
            nc.sync.dma_start(dbg_out["QT0"][:], q_t[0][:])
            nc.sync.dma_start(dbg_out["VA0"][:], v_aug[0][:])
            nc.sync.dma_start(dbg_out["ZT0"][:], z_tt[0][:])

        # ---- output projection (transposed: out^T[fo, q]) ----
        for fo in range(NFT):
            po = ps.tile([128, SQ], F32, tag="a" if fo % 2 == 0 else "b",
                         name=f"po{fo}", padded_shape=[128, SQ])
            for qc in range(SQ // 512):
                qs = slice(512 * qc, 512 * (qc + 1))
                for p4 in range(NFT):
                    nc.tensor.matmul(
                        po[:, qs],
                        x3(wo_s)[:, p4, 128 * fo:128 * (fo + 1)],
                        z_tt[p4][:, qs],
                        start=(p4 == 0),
                        stop=(p4 == NFT - 1),
                    )
            ot = out_pool.tile([128, SQ], F32, tag="ot")
            nc.vector.tensor_scalar_add(ot[:], po[:], bo_all[:, fo:fo + 1])
            nc.sync.dma_start(out[128 * fo:128 * (fo + 1), :], ot[:])

    nc.compile()
    return nc


_NC = None
LAST_RESULTS = None


def _get_nc():
    global _NC
    if _NC is None:
        _NC = build_program(dbg=bool(int(os.environ.get("KERNEL_DEBUG", "0"))))
    return _NC


def _bf(x):
    return np.ascontiguousarray(np.asarray(x, np.float32).astype(
        ml_dtypes.bfloat16))


def _fp8(x):
    return np.ascontiguousarray(
        np.clip(np.asarray(x, np.float32), -240.0, 240.0).astype(
            ml_dtypes.float8_e4m3))


def make_in_maps(Q, K, V, Wq, bq, Wk, bk, Wv, bv, Wo, bo):
    Qf = np.asarray(Q, np.float32)
    Kf = np.asarray(K, np.float32)
    Vf = np.asarray(V, np.float32)
    cast_qk = _fp8 if FP8QK else _bf
    shared = {
        "WQP": cast_qk(np.asarray(Wq, np.float32) * WSCALE),
        "WKP": cast_qk(np.asarray(Wk, np.float32) * WSCALE),
        "WVP": _bf(Wv),
        "WOP": _bf(Wo),
        "bq": np.ascontiguousarray(
            np.asarray(bq, np.float32).reshape(D, 1) * WSCALE),
        "bk": np.ascontiguousarray(
            np.asarray(bk, np.float32).reshape(D, 1) * WSCALE),
        "bv": np.ascontiguousarray(np.asarray(bv, np.float32).reshape(1, D)),
        "bo": np.ascontiguousarray(np.asarray(bo, np.float32).reshape(D, 1)),
    }
    in_maps = []
    for c in range(N_CORES):
        b, qh = c // 2, c % 2
        in_maps.append({
            "XQT": cast_qk(Qf[b, SQ * qh:SQ * (qh + 1)].T),
            "XKT": cast_qk(Kf[b].T),
            "XVT": _bf(Vf[b].T),
            **shared,
        })
    return in_maps


def kernel(Q, K, V, Wq, bq, Wk, bk, Wv, bv, Wo, bo):
    global LAST_RESULTS
    nc = _get_nc()
    in_maps = make_in_maps(Q, K, V, Wq, bq, Wk, bk, Wv, bv, Wo, bo)
    trace = bool(int(os.environ.get("KERNEL_TRACE", "0")))
    res = bass_utils.run_bass_kernel_spmd(
        nc, in_maps, core_ids=list(range(N_CORES)), trace=trace,
    )
    LAST_RESULTS = res
    out = np.empty((B, S, D), dtype=np.float32)
    for c in range(N_CORES):
        b, qh = c // 2, c % 2
        out[b, SQ * qh:SQ * (qh + 1)] = res.results[c]["OUTT"].T
    return out
